# revision 25
# baseline (speedup 1.0000x reference)
"""Block-causal GQA attention for Trainium2, 8 NeuronCores.

Sharding: core = (batch b, GQA group g): 2 batches x 4 kv-groups.
Each core computes its 4 q-heads + 1 kv-head on one batch element in a
"transposed" layout (head_dim on partitions, tokens on free dim), then a
row-parallel partial out-projection; the host sums the 4 partials per batch.

Two macro-phases keep the PE dense (p-state ramp) and the activation table
stable (one Sqrt load, one Exp load):
  Phase A (all chunks): QKV projections (fp16 matmuls), PSUM evacuation on
    the Scalar engine, RMSNorm statistics via PE ones-matmuls, RoPE on DVE
    (fp16, 2x mode), V transposed into v_aug via DMA crossbar.
  Phase B (all chunks): block-sparse scores (fp16), Exp on Scalar (k-norm
    applied via per-partition scale), 0/1 mask multiplies on DVE, PV with an
    augmented ones-row giving softmax denominators, normalize via DVE
    reciprocal + GpSimd partition broadcast, row-parallel out-projection
    software-pipelined one chunk behind attention.
"""
import numpy as np
import ml_dtypes

B, S, DIM = 2, 2048, 1024
H, KVH, HD = 16, 4, 64
EPS = 1e-6
SCALE = HD ** -0.5
PT_TILES = S // 128  # 16
N_CHUNK = 512
N_CHUNKS = S // N_CHUNK  # 4

_BUILD_CACHE = {}
_BLOCKIND = np.zeros((2, 128), np.float32)
_BLOCKIND[0, 0:64] = 1.0
_BLOCKIND[1, 64:128] = 1.0


def _analyze_mask(mask):
    """Classify 128x128 tiles: 0=skip, 1=full, 2=mixed. Returns status grid,
    mixed tile stack (transposed to (k,q) layout, 0/1 float32), and index map.
    Index 0 of the stack is always the all-zero tile."""
    T = PT_TILES
    status = np.zeros((T, T), np.int8)
    tiles = [np.zeros((128, 128), np.float32)]
    idx = {}
    m = np.asarray(mask)
    for i in range(T):
        for j in range(T):
            sub = m[i * 128:(i + 1) * 128, j * 128:(j + 1) * 128]
            if not sub.any():
                status[i, j] = 0
            elif sub.all():
                status[i, j] = 1
            else:
                status[i, j] = 2
                idx[(i, j)] = len(tiles)
                tiles.append(np.ascontiguousarray(sub.T).astype(np.float32))
    return status, np.stack(tiles), idx


def _make_schedule(status, idx):
    """Per chunk: list of (ktile j, s0, s1, [(subtile s, mask_tile_index)])
    where [s0*128, s1*128) is the contiguous span of alive q-subtiles and the
    list holds per-subtile multiplies (zero tile for dead-in-span, mixed id
    for partial)."""
    sched = []
    for ci in range(N_CHUNKS):
        qts = list(range(4 * ci, 4 * ci + 4))
        entries = []
        for j in range(PT_TILES):
            st = [status[i, j] for i in qts]
            if not any(st):
                continue
            alive = [s for s in range(4) if st[s] != 0]
            s0, s1 = alive[0], alive[-1] + 1
            mults = []
            for s in range(s0, s1):
                if st[s] == 1:
                    continue
                mults.append((s, 0 if st[s] == 0 else idx[(qts[s], j)]))
            entries.append((j, s0, s1, mults))
        sched.append(entries)
    return sched


def _build(sched_key, sched, n_masks, neg_c):
    import concourse.bacc as bacc
    import concourse.mybir as mybir
    import concourse.tile as tile

    F32 = mybir.dt.float32
    F32R = mybir.dt.float32r
    F16 = mybir.dt.float16
    BF16 = mybir.dt.bfloat16
    Act = mybir.ActivationFunctionType

    nc = bacc.Bacc("TRN2", target_bir_lowering=False, debug=False)
    xT = nc.dram_tensor("xT", (DIM, S), F16, kind="ExternalInput").ap()
    wq = nc.dram_tensor("wq", (DIM, 256), F16, kind="ExternalInput").ap()
    wkv = nc.dram_tensor("wkv", (DIM, 128), F16, kind="ExternalInput").ap()
    wo = nc.dram_tensor("wo", (256, DIM), BF16, kind="ExternalInput").ap()
    cosq = nc.dram_tensor("cosq", (128, S), F16, kind="ExternalInput").ap()
    sinq = nc.dram_tensor("sinq", (128, S), F16, kind="ExternalInput").ap()
    cosk = nc.dram_tensor("cosk", (64, S), F16, kind="ExternalInput").ap()
    sink = nc.dram_tensor("sink", (64, S), F16, kind="ExternalInput").ap()
    masks = nc.dram_tensor("masks", (n_masks, 128, 128), BF16,
                           kind="ExternalInput").ap()
    blockind_d = nc.dram_tensor("blockind", (2, 128), F32,
                                kind="ExternalInput").ap()
    outT = nc.dram_tensor("outT", (DIM, S), F16, kind="ExternalOutput").ap()

    with tile.TileContext(nc) as tc:
        with tc.tile_pool(name="persist", bufs=1) as pp:
            # --- persistent tiles -------------------------------------
            wq_sb = pp.tile([128, 8, 256], F16)
            nc.sync.dma_start(out=wq_sb, in_=wq.rearrange("(k p) m -> p k m", p=128))
            wkv_sb = pp.tile([128, 8, 128], F16)
            nc.sync.dma_start(out=wkv_sb, in_=wkv.rearrange("(k p) m -> p k m", p=128))
            # masks/wo tiles allocated here; their loads are emitted after
            # chunk-0 input DMAs so phase A starts sooner
            masks_sb = pp.tile([128, n_masks, 128], BF16)
            wo_sb = pp.tile([128, 2, DIM], BF16)

            t1a = [pp.tile([128, S], F16, tag=f"t1a{m}", name=f"t1a{m}") for m in range(2)]
            kt2 = pp.tile([128, S], F16)
            # 80-elem row stride keeps each transpose write 32B-aligned (no RMW races)
            v_aug = pp.tile([128, PT_TILES, 80], BF16)
            rkT = pp.tile([128, 2 * PT_TILES], F32)

            ones1 = pp.tile([128, 1], F32)
            nc.vector.memset(ones1, 1.0)
            nc.vector.tensor_copy(v_aug[:, :, 64:65],
                                  ones1[:].broadcast_to([128, PT_TILES, 1]))
            oq_f = pp.tile([128, 2], F32)
            nc.vector.memset(oq_f, 0.0)
            nc.vector.memset(oq_f[0:64, 0:1], 1.0)
            nc.vector.memset(oq_f[64:128, 1:2], 1.0)
            onesq = pp.tile([128, 2], F16)
            nc.vector.tensor_copy(onesq[:], oq_f[:])
            ok_f = pp.tile([64, 2], F32)
            nc.vector.memset(ok_f, 1.0)
            onesk = pp.tile([64, 2], F16)
            nc.vector.tensor_copy(onesk[:], ok_f[:])
            eps128 = pp.tile([128, 1], F32)
            nc.vector.memset(eps128, EPS)
            bl_f = pp.tile([2, 128], F32)
            nc.sync.dma_start(out=bl_f[:], in_=blockind_d)
            blockind = pp.tile([2, 128], F16)
            nc.vector.tensor_copy(blockind[:], bl_f[:])
            bias_c = pp.tile([128, 1], F32)

            with tc.tile_pool(name="p1", bufs=2) as p1, \
                 tc.tile_pool(name="p2", bufs=4) as p2, \
                 tc.tile_pool(name="p2s", bufs=2) as p2s, \
                 tc.tile_pool(name="psb", bufs=2, space="PSUM") as psb, \
                 tc.tile_pool(name="psv", bufs=4, space="PSUM") as psv:

                # ---- phase A2: norms (small PE + ACT sqrt + DVE recip) --
                def phaseA2(ci, sqq_m, sqk):
                    off = ci * N_CHUNK
                    for m in range(2):
                        nrm_ps = psv.tile([2, N_CHUNK], F32, tag="pv",
                                          name=f"nrm{ci}_{m}")
                        nc.tensor.matmul(nrm_ps[:], onesq[:], sqq_m[m],
                                         start=True, stop=True)
                        nsb = p1.tile([2, N_CHUNK], F32, tag="nsb", name=f"nsb{ci}_{m}")
                        nc.scalar.activation(out=nsb[:], in_=nrm_ps[:],
                                             func=Act.Sqrt,
                                             bias=eps128[0:2], scale=1.0 / HD)
                        nsb2 = p1.tile([2, N_CHUNK], F32, tag="nsb2",
                                       name=f"nsb2{ci}_{m}")
                        nc.vector.reciprocal_approx_fast(out=nsb2[:], in_=nsb[:])
                        nrq = p1.tile([2, N_CHUNK], F16, tag="nrq",
                                      name=f"nrq{ci}_{m}")
                        nc.vector.tensor_copy(nrq[:], nsb2[:])
                        rep_ps = psv.tile([128, N_CHUNK], F32, tag="pv",
                                          name=f"repps{ci}_{m}")
                        nc.tensor.matmul(rep_ps[:], blockind[:], nrq[:],
                                         start=True, stop=True)
                        nc.vector.tensor_mul(
                            t1a[m][:, off:off + N_CHUNK],
                            t1a[m][:, off:off + N_CHUNK], rep_ps[:])
                    nkT_ps = psv.tile([128, 8], F32, tag="pv", name=f"nkT{ci}")
                    for t in range(4):
                        nc.tensor.matmul(nkT_ps[:, 2 * t:2 * t + 2],
                                         sqk[:, t * 128:(t + 1) * 128], onesk[:],
                                         start=(t == 0), stop=(t == 3))
                    rkS = p1.tile([128, 8], F32, tag="rkS", name=f"rkS{ci}")
                    nc.scalar.activation(out=rkS[:], in_=nkT_ps[:],
                                         func=Act.Sqrt,
                                         bias=eps128[:], scale=1.0 / HD)
                    nc.vector.reciprocal_approx_fast(out=rkT[:, 8 * ci:8 * ci + 8],
                                                     in_=rkS[:])

                # ---- phase B: attention -------------------------------
                def phase2(m, ci, fill_cb=None):
                    """Block-sparse attention for (head-pair m, q-chunk ci).
                    fill_cb, if given, is called once per entry to emit the
                    previous chunk's out-projection interleaved for PE density."""
                    off = ci * N_CHUNK
                    entries = sched[ci]
                    attn_c = p2s.tile([128, N_CHUNK], BF16, tag=f"attn{m}",
                                      name=f"attn{m}_{ci}")
                    pv = [psv.tile([65, N_CHUNK], F32, tag="pv", name=f"pv{m}_{ci}_{hh}")
                          for hh in range(2)]
                    n_e = len(entries)
                    sts = [None] * n_e
                    pts = [None] * n_e

                    def emit_scores(ei):
                        j, s0, s1, _ = entries[ei]
                        koff = j * 128
                        a, b_ = s0 * 128, s1 * 128
                        st = psb.tile([128, 2, N_CHUNK], F32, tag="big",
                                      name=f"st{m}_{ci}_{j}")
                        nc.tensor.matmul(
                            st[:, 0, a:b_],
                            kt2[0:64, koff:koff + 128],
                            t1a[m][0:64, off + a:off + b_],
                            start=True, stop=True)
                        nc.tensor.matmul(
                            st[:, 1, a:b_],
                            kt2[64:128, koff:koff + 128],
                            t1a[m][64:128, off + a:off + b_],
                            start=True, stop=True, tile_position=(64, 0))
                        sts[ei] = st

                    emit_scores(0)
                    for ei in range(n_e):
                        j, s0, s1, mults = entries[ei]
                        a, b_ = s0 * 128, s1 * 128
                        if ei + 1 < n_e:
                            emit_scores(ei + 1)
                        st = sts[ei]
                        pt = p2.tile([128, 2, N_CHUNK], BF16, tag="pt",
                                     name=f"pt{m}_{ci}_{j}")
                        nc.scalar.activation(
                            out=pt[:, :, a:b_], in_=st[:, :, a:b_],
                            func=Act.Exp,
                            bias=bias_c[:], scale=rkT[:, 2 * j:2 * j + 1])
                        for s_, mt in mults:
                            for hh in range(2):
                                nc.vector.tensor_mul(
                                    pt[:, hh, s_ * 128:(s_ + 1) * 128],
                                    pt[:, hh, s_ * 128:(s_ + 1) * 128],
                                    masks_sb[:, mt, :])
                        first = (ei == 0)
                        last = (ei == n_e - 1)
                        for hh in range(2):
                            nc.tensor.matmul(pv[hh][:, a:b_],
                                             v_aug[:, j, 0:65],
                                             pt[:, hh, a:b_],
                                             start=first, stop=last)
                        if fill_cb is not None:
                            fill_cb(ei)
                    # softmax normalize: 1/denominator broadcast to 64 rows
                    dsb = p2s.tile([1, 2, N_CHUNK], F32, tag="dsb", name=f"dsb{m}_{ci}")
                    for hh in range(2):
                        nc.vector.tensor_copy(dsb[:, hh, :], pv[hh][64:65, :])
                    rd = p2s.tile([1, 2, N_CHUNK], F32, tag="rd", name=f"rd{m}_{ci}")
                    nc.vector.reciprocal_approx_fast(out=rd[:], in_=dsb[:])
                    bcd = p2s.tile([64, 2, N_CHUNK], F32, tag="bcd",
                                   name=f"bcd{m}_{ci}")
                    nc.gpsimd.partition_broadcast(bcd[:], rd[:], channels=64)
                    for hh in range(2):
                        nc.vector.tensor_mul(
                            attn_c[hh * 64:(hh + 1) * 64, :],
                            pv[hh][0:64, :], bcd[:, hh, :])
                    return attn_c

                def make_phase3_cb(ci, attn_ts):
                    """Returns a callback emitting one out-proj column tile per
                    invocation, interleaved into the following phase2."""
                    off = ci * N_CHUNK
                    state = {"mo": 0}

                    def cb(_ei):
                        mo = state["mo"]
                        if mo >= 8:
                            return
                        state["mo"] += 1
                        o_ps = psb.tile([128, 2, N_CHUNK], F32, tag="big",
                                        name=f"ops{ci}_{mo}")
                        for k2_ in range(2):
                            nc.tensor.matmul(o_ps[:, 0, :],
                                             wo_sb[:, k2_, mo * 128:(mo + 1) * 128],
                                             attn_ts[k2_][:],
                                             start=(k2_ == 0), stop=(k2_ == 1))
                        o_sb = p1.tile([128, N_CHUNK], F16, tag="osb",
                                       bufs=3, name=f"osb{ci}_{mo}")
                        nc.vector.tensor_copy(o_sb[:], o_ps[:, 0, :])
                        nc.scalar.dma_start(
                            out=outT[mo * 128:(mo + 1) * 128, off:off + N_CHUNK],
                            in_=o_sb[:])

                    def flush():
                        while state["mo"] < 8:
                            cb(None)
                    cb.flush = flush
                    return cb

                # ---- phase A1: projections + rope (PE dense) ----------
                def phaseA1_and_track(ci):
                    off = ci * N_CHUNK
                    sqq_m = []
                    xt = p1.tile([128, 8, N_CHUNK], F16, tag="xt", name=f"xt{ci}")
                    nc.sync.dma_start(
                        out=xt,
                        in_=xT[:, off:off + N_CHUNK].rearrange("(k p) n -> p k n", p=128))
                    cq = p1.tile([128, N_CHUNK], F16, tag="cq", name=f"cq{ci}")
                    nc.sync.dma_start(out=cq, in_=cosq[:, off:off + N_CHUNK])
                    sq = p1.tile([128, N_CHUNK], F16, tag="sq", name=f"sq{ci}")
                    nc.sync.dma_start(out=sq, in_=sinq[:, off:off + N_CHUNK])
                    ck = p1.tile([64, N_CHUNK], F16, tag="ck", name=f"ck{ci}")
                    nc.sync.dma_start(out=ck, in_=cosk[:, off:off + N_CHUNK])
                    sk = p1.tile([64, N_CHUNK], F16, tag="sk", name=f"sk{ci}")
                    nc.sync.dma_start(out=sk, in_=sink[:, off:off + N_CHUNK])

                    for m in range(2):
                        q_ps = psb.tile([128, 2, N_CHUNK], F32, tag="big",
                                        name=f"qps{ci}_{m}")
                        for k in range(8):
                            nc.tensor.matmul(q_ps[:, 0, :],
                                             wq_sb[:, k, m * 128:(m + 1) * 128],
                                             xt[:, k, :],
                                             start=(k == 0), stop=(k == 7))
                        qtr = p1.tile([128, N_CHUNK], F16, tag="qtr", name=f"qtr{ci}_{m}")
                        nc.scalar.copy(qtr[:], q_ps[:, 0, :])
                        sqq = p1.tile([128, N_CHUNK], F16, tag="sqq", bufs=4,
                                      name=f"sqq{ci}_{m}")
                        nc.scalar.activation(out=sqq[:], in_=q_ps[:, 0, :],
                                             func=Act.Square)
                        sqq_m.append(sqq)
                        qrot = p1.tile([128, N_CHUNK], F16, tag="qrot", name=f"qrot{ci}_{m}")
                        for blk, srcp in enumerate((32, 0, 96, 64)):
                            nc.vector.tensor_copy(qrot[blk * 32:(blk + 1) * 32, :],
                                                  qtr[srcp:srcp + 32, :])
                        tq = p1.tile([128, N_CHUNK], F16, tag="tq", name=f"tq{ci}_{m}")
                        nc.vector.tensor_mul(tq[:], qtr[:], cq[:])
                        nc.vector.tensor_mul(qrot[:], qrot[:], sq[:])
                        nc.vector.tensor_add(t1a[m][:, off:off + N_CHUNK],
                                             tq[:], qrot[:])

                    kv_ps = psb.tile([128, 2, N_CHUNK], F32, tag="big",
                                     name=f"kvps{ci}")
                    for k in range(8):
                        nc.tensor.matmul(kv_ps[:, 0, :], wkv_sb[:, k, :], xt[:, k, :],
                                         start=(k == 0), stop=(k == 7))
                    ktr = p1.tile([64, N_CHUNK], F16, tag="ktr", name=f"ktr{ci}")
                    nc.scalar.copy(ktr[:], kv_ps[0:64, 0, :])
                    vtr = p1.tile([64, N_CHUNK], BF16, tag="vtr", name=f"vtr{ci}")
                    nc.vector.tensor_copy(vtr[:], kv_ps[64:128, 0, :])
                    sqk = p1.tile([64, N_CHUNK], F16, tag="sqk", name=f"sqk{ci}")
                    nc.scalar.activation(out=sqk[:], in_=kv_ps[0:64, 0, :],
                                         func=Act.Square)
                    krot = p1.tile([64, N_CHUNK], F16, tag="krot", name=f"krot{ci}")
                    nc.vector.tensor_copy(krot[0:32, :], ktr[32:64, :])
                    nc.vector.tensor_copy(krot[32:64, :], ktr[0:32, :])
                    k1 = p1.tile([64, N_CHUNK], F16, tag="k1", name=f"k1{ci}")
                    nc.vector.tensor_mul(k1[:], ktr[:], ck[:])
                    nc.vector.tensor_mul(krot[:], krot[:], sk[:])
                    nc.vector.tensor_add(kt2[0:64, off:off + N_CHUNK], k1[:], krot[:])
                    nc.vector.tensor_copy(kt2[64:128, off:off + N_CHUNK],
                                          kt2[0:64, off:off + N_CHUNK])
                    for t in range(4):
                        j = 4 * ci + t
                        nc.scalar.dma_start_transpose(
                            out=v_aug[:, j, 0:64],
                            in_=vtr[:, t * 128:(t + 1) * 128])
                    return sqq_m, sqk

                a_state = {}
                a_state[0] = phaseA1_and_track(0)
                # phase-B-only loads, deferred past chunk 0's inputs
                nc.sync.dma_start(out=masks_sb,
                                  in_=masks.rearrange("n k q -> k n q"))
                a_state[1] = phaseA1_and_track(1)
                nc.sync.dma_start(out=wo_sb,
                                  in_=wo.rearrange("(k p) m -> p k m", p=128))
                phaseA2(0, *a_state.pop(0))
                a_state[2] = phaseA1_and_track(2)
                phaseA2(1, *a_state.pop(1))
                a_state[3] = phaseA1_and_track(3)
                phaseA2(2, *a_state.pop(2))
                phaseA2(3, *a_state.pop(3))
                # bias_c = rkT[:,31]*0 + neg_c: every Exp reads bias_c, so this
                # forces all phase-A sqrts to schedule before any exp (the ACT
                # table is swapped exactly once)
                nc.vector.tensor_scalar(
                    out=bias_c[:], in0=rkT[:, 31:32], scalar1=0.0,
                    scalar2=neg_c, op0=mybir.AluOpType.mult,
                    op1=mybir.AluOpType.add)

                # Phase B with out-projection pipelined one chunk behind
                attn_prev = None
                for ci in range(N_CHUNKS):
                    a0 = phase2(0, ci)
                    cb = None
                    if ci > 0:
                        cb = make_phase3_cb(ci - 1, attn_prev)
                    a1 = phase2(1, ci, fill_cb=cb)
                    if cb is not None:
                        cb.flush()
                    attn_prev = (a0, a1)
                last_cb = make_phase3_cb(N_CHUNKS - 1, attn_prev)
                last_cb.flush()

    nc.compile()
    return nc


def _get_nc(sched_key, sched, n_masks, neg_c):
    key = (sched_key, n_masks, float(neg_c))
    if key not in _BUILD_CACHE:
        _BUILD_CACHE[key] = _build(sched_key, sched, n_masks, neg_c)
    return _BUILD_CACHE[key]


def kernel(x, Wq, Wkv, Wo, q_norm_w, k_norm_w, rope_cos, rope_sin,
           attention_mask):
    x = np.asarray(x, dtype=np.float32)
    Wq = np.asarray(Wq, dtype=np.float32)
    Wkv = np.asarray(Wkv, dtype=np.float32)
    Wo = np.asarray(Wo, dtype=np.float32)
    qw = np.asarray(q_norm_w, dtype=np.float32)
    kw = np.asarray(k_norm_w, dtype=np.float32)
    cos = np.asarray(rope_cos, dtype=np.float32)
    sin = np.asarray(rope_sin, dtype=np.float32)

    status, mask_tiles, idx = _analyze_mask(attention_mask)
    sched = _make_schedule(status, idx)
    sched_key = status.tobytes()

    # numerically safe exp shift (0 in the normal regime)
    mct_q = max(np.abs(cos).max(), np.abs(sin).max(), 1e-9)
    bound = SCALE * 2.0 * HD * mct_q * mct_q \
        * max(np.abs(qw).max(), 1e-9) * max(np.abs(kw).max(), 1e-9)
    neg_c = -max(0.0, float(bound) - 60.0)

    nc = _get_nc(sched_key, sched, mask_tiles.shape[0], neg_c)

    # host-folded rope tables (transposed layout, head-dim on partitions)
    half = HD // 2
    swap = np.concatenate([np.arange(half, HD), np.arange(0, half)])
    sgn = np.concatenate([-np.ones(half, np.float32), np.ones(half, np.float32)])
    cosq_h = (cos.T * qw[:, None] * SCALE).astype(np.float16)          # (64, S)
    sinq_h = (sin.T * (sgn * qw[swap])[:, None] * SCALE).astype(np.float16)
    cosk_h = (cos.T * kw[:, None]).astype(np.float16)
    sink_h = (sin.T * (sgn * kw[swap])[:, None]).astype(np.float16)
    cosq2 = np.ascontiguousarray(np.concatenate([cosq_h, cosq_h], axis=0))
    sinq2 = np.ascontiguousarray(np.concatenate([sinq_h, sinq_h], axis=0))

    in_maps = []
    for c in range(8):
        b, g = c // 4, c % 4
        im = {
            "xT": np.ascontiguousarray(x[b].T).astype(np.float16),
            "wq": np.ascontiguousarray(Wq[:, g * 256:(g + 1) * 256]).astype(np.float16),
            "wkv": np.ascontiguousarray(
                np.concatenate([Wkv[:, g * HD:(g + 1) * HD],
                                Wkv[:, KVH * HD + g * HD: KVH * HD + (g + 1) * HD]],
                               axis=1)).astype(np.float16),
            "wo": np.ascontiguousarray(Wo[g * 256:(g + 1) * 256, :]).astype(ml_dtypes.bfloat16),
            "cosq": cosq2, "sinq": sinq2,
            "cosk": np.ascontiguousarray(cosk_h),
            "sink": np.ascontiguousarray(sink_h),
            "masks": mask_tiles.astype(ml_dtypes.bfloat16),
            "blockind": _BLOCKIND,
        }
        in_maps.append(im)

    from concourse.bass_utils import run_bass_kernel_spmd
    res = run_bass_kernel_spmd(nc, in_maps, core_ids=list(range(8)), trace=False)

    out = np.zeros((B, S, DIM), dtype=np.float32)
    for c in range(8):
        out[c // 4] += res.results[c]["outT"].T.astype(np.float32)
    return out


# revision 26
# speedup vs baseline: 1.2201x; 1.2201x over previous
"""Block-causal GQA attention for Trainium2, 8 NeuronCores.

Sharding: core = (batch b, GQA group g): 2 batches x 4 kv-groups.
Each core computes its 4 q-heads + 1 kv-head on one batch element in a
"transposed" layout (head_dim on partitions, tokens on free dim), then a
row-parallel partial out-projection; the host sums the 4 partials per batch.

Two macro-phases keep the PE dense (p-state ramp) and the activation table
stable (one Sqrt load, one Exp load):
  Phase A (all chunks): QKV projections (fp16 matmuls), PSUM evacuation on
    the Scalar engine, RMSNorm statistics via PE ones-matmuls, RoPE on DVE
    (fp16, 2x mode), V transposed into v_aug via DMA crossbar.
  Phase B (all chunks): block-sparse scores (fp16), Exp on Scalar (k-norm
    applied via per-partition scale), 0/1 mask multiplies on DVE, PV with an
    augmented ones-row giving softmax denominators, normalize via DVE
    reciprocal + GpSimd partition broadcast, row-parallel out-projection
    software-pipelined one chunk behind attention.
"""
import numpy as np
import ml_dtypes

B, S, DIM = 2, 2048, 1024
H, KVH, HD = 16, 4, 64
EPS = 1e-6
SCALE = HD ** -0.5
PT_TILES = S // 128  # 16
N_CHUNK = 512
N_CHUNKS = S // N_CHUNK  # 4

_BUILD_CACHE = {}
_BLOCKIND = np.zeros((2, 128), np.float32)
_BLOCKIND[0, 0:64] = 1.0
_BLOCKIND[1, 64:128] = 1.0


def _analyze_mask(mask):
    """Classify 128x128 tiles: 0=skip, 1=full, 2=mixed. Returns status grid,
    mixed tile stack (transposed to (k,q) layout, 0/1 float32), and index map.
    Index 0 of the stack is always the all-zero tile."""
    T = PT_TILES
    status = np.zeros((T, T), np.int8)
    tiles = [np.zeros((128, 128), np.float32)]
    idx = {}
    m = np.asarray(mask)
    for i in range(T):
        for j in range(T):
            sub = m[i * 128:(i + 1) * 128, j * 128:(j + 1) * 128]
            if not sub.any():
                status[i, j] = 0
            elif sub.all():
                status[i, j] = 1
            else:
                status[i, j] = 2
                idx[(i, j)] = len(tiles)
                tiles.append(np.ascontiguousarray(sub.T).astype(np.float32))
    return status, np.stack(tiles), idx


def _make_schedule(status, idx):
    """Per chunk: list of (ktile j, s0, s1, [(subtile s, mask_tile_index)])
    where [s0*128, s1*128) is the contiguous span of alive q-subtiles and the
    list holds per-subtile multiplies (zero tile for dead-in-span, mixed id
    for partial)."""
    sched = []
    for ci in range(N_CHUNKS):
        qts = list(range(4 * ci, 4 * ci + 4))
        entries = []
        for j in range(PT_TILES):
            st = [status[i, j] for i in qts]
            if not any(st):
                continue
            alive = [s for s in range(4) if st[s] != 0]
            s0, s1 = alive[0], alive[-1] + 1
            mults = []
            for s in range(s0, s1):
                if st[s] == 1:
                    continue
                mults.append((s, 0 if st[s] == 0 else idx[(qts[s], j)]))
            entries.append((j, s0, s1, mults))
        sched.append(entries)
    return sched


def _build(sched_key, sched, n_masks, neg_c):
    import concourse.bacc as bacc
    import concourse.mybir as mybir
    import concourse.tile as tile

    F32 = mybir.dt.float32
    F32R = mybir.dt.float32r
    F16 = mybir.dt.float16
    BF16 = mybir.dt.bfloat16
    Act = mybir.ActivationFunctionType

    nc = bacc.Bacc("TRN2", target_bir_lowering=False, debug=False)
    xT = nc.dram_tensor("xT", (DIM, S), F16, kind="ExternalInput").ap()
    wq = nc.dram_tensor("wq", (DIM, 256), F16, kind="ExternalInput").ap()
    wkv = nc.dram_tensor("wkv", (DIM, 128), F16, kind="ExternalInput").ap()
    wo = nc.dram_tensor("wo", (256, DIM), BF16, kind="ExternalInput").ap()
    cosq = nc.dram_tensor("cosq", (128, S), F16, kind="ExternalInput").ap()
    sinq = nc.dram_tensor("sinq", (128, S), F16, kind="ExternalInput").ap()
    cosk = nc.dram_tensor("cosk", (64, S), F16, kind="ExternalInput").ap()
    sink = nc.dram_tensor("sink", (64, S), F16, kind="ExternalInput").ap()
    masks = nc.dram_tensor("masks", (n_masks, 128, 128), BF16,
                           kind="ExternalInput").ap()
    blockind_d = nc.dram_tensor("blockind", (2, 128), F32,
                                kind="ExternalInput").ap()
    outT = nc.dram_tensor("outT", (DIM, S), F16, kind="ExternalOutput").ap()

    with tile.TileContext(nc) as tc:
        with tc.tile_pool(name="persist", bufs=1) as pp:
            # --- persistent tiles -------------------------------------
            wq_sb = pp.tile([128, 8, 256], F16)
            nc.sync.dma_start(out=wq_sb, in_=wq.rearrange("(k p) m -> p k m", p=128))
            wkv_sb = pp.tile([128, 8, 128], F16)
            nc.sync.dma_start(out=wkv_sb, in_=wkv.rearrange("(k p) m -> p k m", p=128))
            # masks/wo tiles allocated here; their loads are emitted after
            # chunk-0 input DMAs so phase A starts sooner
            masks_sb = pp.tile([128, n_masks, 128], BF16)
            wo_sb = pp.tile([128, 2, DIM], BF16)

            t1a = [pp.tile([128, S], F16, tag=f"t1a{m}", name=f"t1a{m}") for m in range(2)]
            kt2 = pp.tile([128, S], F16)
            # 80-elem row stride keeps each transpose write 32B-aligned (no RMW races)
            v_aug = pp.tile([128, PT_TILES, 80], BF16)
            rkT = pp.tile([128, 2 * PT_TILES], F32)

            ones1 = pp.tile([128, 1], F32)
            nc.vector.memset(ones1, 1.0)
            nc.vector.tensor_copy(v_aug[:, :, 64:65],
                                  ones1[:].broadcast_to([128, PT_TILES, 1]))
            oq_f = pp.tile([128, 2], F32)
            nc.vector.memset(oq_f, 0.0)
            nc.vector.memset(oq_f[0:64, 0:1], 1.0)
            nc.vector.memset(oq_f[64:128, 1:2], 1.0)
            onesq = pp.tile([128, 2], F16)
            nc.vector.tensor_copy(onesq[:], oq_f[:])
            ok_f = pp.tile([64, 2], F32)
            nc.vector.memset(ok_f, 1.0)
            onesk = pp.tile([64, 2], F16)
            nc.vector.tensor_copy(onesk[:], ok_f[:])
            eps128 = pp.tile([128, 1], F32)
            nc.vector.memset(eps128, EPS)
            bl_f = pp.tile([2, 128], F32)
            nc.sync.dma_start(out=bl_f[:], in_=blockind_d)
            blockind = pp.tile([2, 128], F16)
            nc.vector.tensor_copy(blockind[:], bl_f[:])
            bias_c = pp.tile([128, 1], F32)

            with tc.tile_pool(name="p1", bufs=2) as p1, \
                 tc.tile_pool(name="p2", bufs=4) as p2, \
                 tc.tile_pool(name="p2s", bufs=2) as p2s, \
                 tc.tile_pool(name="psb", bufs=2, space="PSUM") as psb, \
                 tc.tile_pool(name="psv", bufs=4, space="PSUM") as psv:

                # ---- phase A2: norms (small PE + ACT sqrt + DVE recip) --
                def phaseA2(ci, sqq_m, sqk):
                    off = ci * N_CHUNK
                    for m in range(2):
                        nrm_ps = psv.tile([2, N_CHUNK], F32, tag="pv",
                                          name=f"nrm{ci}_{m}")
                        nc.tensor.matmul(nrm_ps[:], onesq[:], sqq_m[m],
                                         start=True, stop=True)
                        nsb = p1.tile([2, N_CHUNK], F32, tag="nsb", name=f"nsb{ci}_{m}")
                        nc.scalar.activation(out=nsb[:], in_=nrm_ps[:],
                                             func=Act.Sqrt,
                                             bias=eps128[0:2], scale=1.0 / HD)
                        nsb2 = p1.tile([2, N_CHUNK], F32, tag="nsb2",
                                       name=f"nsb2{ci}_{m}")
                        nc.vector.reciprocal_approx_fast(out=nsb2[:], in_=nsb[:])
                        nrq = p1.tile([2, N_CHUNK], F16, tag="nrq",
                                      name=f"nrq{ci}_{m}")
                        nc.vector.tensor_copy(nrq[:], nsb2[:])
                        rep_ps = psv.tile([128, N_CHUNK], F32, tag="pv",
                                          name=f"repps{ci}_{m}")
                        nc.tensor.matmul(rep_ps[:], blockind[:], nrq[:],
                                         start=True, stop=True)
                        nc.vector.tensor_mul(
                            t1a[m][:, off:off + N_CHUNK],
                            t1a[m][:, off:off + N_CHUNK], rep_ps[:])
                    nkT_ps = psv.tile([128, 8], F32, tag="pv", name=f"nkT{ci}")
                    for t in range(4):
                        nc.tensor.matmul(nkT_ps[:, 2 * t:2 * t + 2],
                                         sqk[:, t * 128:(t + 1) * 128], onesk[:],
                                         start=(t == 0), stop=(t == 3))
                    rkS = p1.tile([128, 8], F32, tag="rkS", name=f"rkS{ci}")
                    nc.scalar.activation(out=rkS[:], in_=nkT_ps[:],
                                         func=Act.Sqrt,
                                         bias=eps128[:], scale=1.0 / HD)
                    nc.vector.reciprocal_approx_fast(out=rkT[:, 8 * ci:8 * ci + 8],
                                                     in_=rkS[:])

                # ---- phase B: attention -------------------------------
                def phase2(m, ci, fill_cb=None):
                    """Block-sparse attention for (head-pair m, q-chunk ci).
                    fill_cb, if given, is called once per entry to emit the
                    previous chunk's out-projection interleaved for PE density."""
                    off = ci * N_CHUNK
                    entries = sched[ci]
                    attn_c = p2s.tile([128, N_CHUNK], BF16, tag=f"attn{m}",
                                      name=f"attn{m}_{ci}")
                    pv = [psv.tile([65, N_CHUNK], F32, tag="pv", name=f"pv{m}_{ci}_{hh}")
                          for hh in range(2)]
                    n_e = len(entries)
                    sts = [None] * n_e
                    pts = [None] * n_e

                    def emit_scores(ei):
                        j, s0, s1, _ = entries[ei]
                        koff = j * 128
                        a, b_ = s0 * 128, s1 * 128
                        st = psb.tile([128, 2, N_CHUNK], F32, tag="big",
                                      name=f"st{m}_{ci}_{j}")
                        nc.tensor.matmul(
                            st[:, 0, a:b_],
                            kt2[0:64, koff:koff + 128],
                            t1a[m][0:64, off + a:off + b_],
                            start=True, stop=True)
                        nc.tensor.matmul(
                            st[:, 1, a:b_],
                            kt2[64:128, koff:koff + 128],
                            t1a[m][64:128, off + a:off + b_],
                            start=True, stop=True, tile_position=(64, 0))
                        sts[ei] = st

                    emit_scores(0)
                    for ei in range(n_e):
                        j, s0, s1, mults = entries[ei]
                        a, b_ = s0 * 128, s1 * 128
                        if ei + 1 < n_e:
                            emit_scores(ei + 1)
                        st = sts[ei]
                        pt = p2.tile([128, 2, N_CHUNK], BF16, tag="pt",
                                     name=f"pt{m}_{ci}_{j}")
                        nc.scalar.activation(
                            out=pt[:, :, a:b_], in_=st[:, :, a:b_],
                            func=Act.Exp,
                            bias=bias_c[:], scale=rkT[:, 2 * j:2 * j + 1])
                        for s_, mt in mults:
                            for hh in range(2):
                                nc.vector.tensor_mul(
                                    pt[:, hh, s_ * 128:(s_ + 1) * 128],
                                    pt[:, hh, s_ * 128:(s_ + 1) * 128],
                                    masks_sb[:, mt, :])
                        first = (ei == 0)
                        last = (ei == n_e - 1)
                        for hh in range(2):
                            nc.tensor.matmul(pv[hh][:, a:b_],
                                             v_aug[:, j, 0:65],
                                             pt[:, hh, a:b_],
                                             start=first, stop=last)
                        if fill_cb is not None:
                            fill_cb(ei)
                    # softmax normalize: 1/denominator broadcast to 64 rows
                    dsb = p2s.tile([1, 2, N_CHUNK], F32, tag="dsb", name=f"dsb{m}_{ci}")
                    for hh in range(2):
                        nc.vector.tensor_copy(dsb[:, hh, :], pv[hh][64:65, :])
                    rd = p2s.tile([1, 2, N_CHUNK], F32, tag="rd", name=f"rd{m}_{ci}")
                    nc.vector.reciprocal_approx_fast(out=rd[:], in_=dsb[:])
                    bcd = p2s.tile([64, 2, N_CHUNK], F32, tag="bcd",
                                   name=f"bcd{m}_{ci}")
                    nc.gpsimd.partition_broadcast(bcd[:], rd[:], channels=64)
                    for hh in range(2):
                        nc.vector.tensor_mul(
                            attn_c[hh * 64:(hh + 1) * 64, :],
                            pv[hh][0:64, :], bcd[:, hh, :])
                    return attn_c

                def make_phase3_cb(ci, attn_ts):
                    """Returns a callback emitting one out-proj column tile per
                    invocation, interleaved into the following phase2."""
                    off = ci * N_CHUNK
                    state = {"mo": 0}

                    def cb(_ei):
                        mo = state["mo"]
                        if mo >= 8:
                            return
                        state["mo"] += 1
                        o_ps = psb.tile([128, 2, N_CHUNK], F32, tag="big",
                                        name=f"ops{ci}_{mo}")
                        for k2_ in range(2):
                            nc.tensor.matmul(o_ps[:, 0, :],
                                             wo_sb[:, k2_, mo * 128:(mo + 1) * 128],
                                             attn_ts[k2_][:],
                                             start=(k2_ == 0), stop=(k2_ == 1))
                        o_sb = p1.tile([128, N_CHUNK], F16, tag="osb",
                                       bufs=3, name=f"osb{ci}_{mo}")
                        nc.vector.tensor_copy(o_sb[:], o_ps[:, 0, :])
                        nc.scalar.dma_start(
                            out=outT[mo * 128:(mo + 1) * 128, off:off + N_CHUNK],
                            in_=o_sb[:])

                    def flush():
                        while state["mo"] < 8:
                            cb(None)
                    cb.flush = flush
                    return cb

                # ---- phase A1: projections + rope (PE dense) ----------
                def phaseA1_and_track(ci):
                    off = ci * N_CHUNK
                    sqq_m = []
                    xt = p1.tile([128, 8, N_CHUNK], F16, tag="xt", name=f"xt{ci}")
                    nc.sync.dma_start(
                        out=xt,
                        in_=xT[:, off:off + N_CHUNK].rearrange("(k p) n -> p k n", p=128))
                    cq = p1.tile([128, N_CHUNK], F16, tag="cq", name=f"cq{ci}")
                    nc.sync.dma_start(out=cq, in_=cosq[:, off:off + N_CHUNK])
                    sq = p1.tile([128, N_CHUNK], F16, tag="sq", name=f"sq{ci}")
                    nc.sync.dma_start(out=sq, in_=sinq[:, off:off + N_CHUNK])
                    ck = p1.tile([64, N_CHUNK], F16, tag="ck", name=f"ck{ci}")
                    nc.sync.dma_start(out=ck, in_=cosk[:, off:off + N_CHUNK])
                    sk = p1.tile([64, N_CHUNK], F16, tag="sk", name=f"sk{ci}")
                    nc.sync.dma_start(out=sk, in_=sink[:, off:off + N_CHUNK])

                    for m in range(2):
                        q_ps = psb.tile([128, 2, N_CHUNK], F32, tag="big",
                                        name=f"qps{ci}_{m}")
                        for k in range(8):
                            nc.tensor.matmul(q_ps[:, 0, :],
                                             wq_sb[:, k, m * 128:(m + 1) * 128],
                                             xt[:, k, :],
                                             start=(k == 0), stop=(k == 7))
                        qtr = p1.tile([128, N_CHUNK], F16, tag="qtr", name=f"qtr{ci}_{m}")
                        nc.scalar.copy(qtr[:], q_ps[:, 0, :])
                        sqq = p1.tile([128, N_CHUNK], F16, tag="sqq", bufs=4,
                                      name=f"sqq{ci}_{m}")
                        nc.scalar.activation(out=sqq[:], in_=q_ps[:, 0, :],
                                             func=Act.Square)
                        sqq_m.append(sqq)
                        qrot = p1.tile([128, N_CHUNK], F16, tag="qrot", name=f"qrot{ci}_{m}")
                        for blk, srcp in enumerate((32, 0, 96, 64)):
                            nc.vector.tensor_copy(qrot[blk * 32:(blk + 1) * 32, :],
                                                  qtr[srcp:srcp + 32, :])
                        tq = p1.tile([128, N_CHUNK], F16, tag="tq", name=f"tq{ci}_{m}")
                        nc.vector.tensor_mul(tq[:], qtr[:], cq[:])
                        nc.vector.tensor_mul(qrot[:], qrot[:], sq[:])
                        nc.vector.tensor_add(t1a[m][:, off:off + N_CHUNK],
                                             tq[:], qrot[:])

                    kv_ps = psb.tile([128, 2, N_CHUNK], F32, tag="big",
                                     name=f"kvps{ci}")
                    for k in range(8):
                        nc.tensor.matmul(kv_ps[:, 0, :], wkv_sb[:, k, :], xt[:, k, :],
                                         start=(k == 0), stop=(k == 7))
                    ktr = p1.tile([64, N_CHUNK], F16, tag="ktr", name=f"ktr{ci}")
                    nc.scalar.copy(ktr[:], kv_ps[0:64, 0, :])
                    vtr = p1.tile([64, N_CHUNK], BF16, tag="vtr", name=f"vtr{ci}")
                    nc.vector.tensor_copy(vtr[:], kv_ps[64:128, 0, :])
                    sqk = p1.tile([64, N_CHUNK], F16, tag="sqk", name=f"sqk{ci}")
                    nc.scalar.activation(out=sqk[:], in_=kv_ps[0:64, 0, :],
                                         func=Act.Square)
                    krot = p1.tile([64, N_CHUNK], F16, tag="krot", name=f"krot{ci}")
                    nc.vector.tensor_copy(krot[0:32, :], ktr[32:64, :])
                    nc.vector.tensor_copy(krot[32:64, :], ktr[0:32, :])
                    k1 = p1.tile([64, N_CHUNK], F16, tag="k1", name=f"k1{ci}")
                    nc.vector.tensor_mul(k1[:], ktr[:], ck[:])
                    nc.vector.tensor_mul(krot[:], krot[:], sk[:])
                    nc.vector.tensor_add(kt2[0:64, off:off + N_CHUNK], k1[:], krot[:])
                    nc.vector.tensor_copy(kt2[64:128, off:off + N_CHUNK],
                                          kt2[0:64, off:off + N_CHUNK])
                    for t in range(4):
                        j = 4 * ci + t
                        nc.sync.dma_start_transpose(
                            out=v_aug[:, j, 0:64],
                            in_=vtr[:, t * 128:(t + 1) * 128])
                    return sqq_m, sqk

                a_state = {}
                a_state[0] = phaseA1_and_track(0)
                # phase-B-only loads, deferred past chunk 0's inputs
                nc.sync.dma_start(out=masks_sb,
                                  in_=masks.rearrange("n k q -> k n q"))
                a_state[1] = phaseA1_and_track(1)
                nc.sync.dma_start(out=wo_sb,
                                  in_=wo.rearrange("(k p) m -> p k m", p=128))
                phaseA2(0, *a_state.pop(0))
                a_state[2] = phaseA1_and_track(2)
                phaseA2(1, *a_state.pop(1))
                a_state[3] = phaseA1_and_track(3)
                phaseA2(2, *a_state.pop(2))
                phaseA2(3, *a_state.pop(3))
                # bias_c = rkT[:,31]*0 + neg_c: every Exp reads bias_c, so this
                # forces all phase-A sqrts to schedule before any exp (the ACT
                # table is swapped exactly once)
                nc.vector.tensor_scalar(
                    out=bias_c[:], in0=rkT[:, 31:32], scalar1=0.0,
                    scalar2=neg_c, op0=mybir.AluOpType.mult,
                    op1=mybir.AluOpType.add)

                # Phase B with out-projection pipelined one chunk behind
                attn_prev = None
                for ci in range(N_CHUNKS):
                    a0 = phase2(0, ci)
                    cb = None
                    if ci > 0:
                        cb = make_phase3_cb(ci - 1, attn_prev)
                    a1 = phase2(1, ci, fill_cb=cb)
                    if cb is not None:
                        cb.flush()
                    attn_prev = (a0, a1)
                last_cb = make_phase3_cb(N_CHUNKS - 1, attn_prev)
                last_cb.flush()

    nc.compile()
    return nc


def _get_nc(sched_key, sched, n_masks, neg_c):
    key = (sched_key, n_masks, float(neg_c))
    if key not in _BUILD_CACHE:
        _BUILD_CACHE[key] = _build(sched_key, sched, n_masks, neg_c)
    return _BUILD_CACHE[key]


def kernel(x, Wq, Wkv, Wo, q_norm_w, k_norm_w, rope_cos, rope_sin,
           attention_mask):
    x = np.asarray(x, dtype=np.float32)
    Wq = np.asarray(Wq, dtype=np.float32)
    Wkv = np.asarray(Wkv, dtype=np.float32)
    Wo = np.asarray(Wo, dtype=np.float32)
    qw = np.asarray(q_norm_w, dtype=np.float32)
    kw = np.asarray(k_norm_w, dtype=np.float32)
    cos = np.asarray(rope_cos, dtype=np.float32)
    sin = np.asarray(rope_sin, dtype=np.float32)

    status, mask_tiles, idx = _analyze_mask(attention_mask)
    sched = _make_schedule(status, idx)
    sched_key = status.tobytes()

    # numerically safe exp shift (0 in the normal regime)
    mct_q = max(np.abs(cos).max(), np.abs(sin).max(), 1e-9)
    bound = SCALE * 2.0 * HD * mct_q * mct_q \
        * max(np.abs(qw).max(), 1e-9) * max(np.abs(kw).max(), 1e-9)
    neg_c = -max(0.0, float(bound) - 60.0)

    nc = _get_nc(sched_key, sched, mask_tiles.shape[0], neg_c)

    # host-folded rope tables (transposed layout, head-dim on partitions)
    half = HD // 2
    swap = np.concatenate([np.arange(half, HD), np.arange(0, half)])
    sgn = np.concatenate([-np.ones(half, np.float32), np.ones(half, np.float32)])
    cosq_h = (cos.T * qw[:, None] * SCALE).astype(np.float16)          # (64, S)
    sinq_h = (sin.T * (sgn * qw[swap])[:, None] * SCALE).astype(np.float16)
    cosk_h = (cos.T * kw[:, None]).astype(np.float16)
    sink_h = (sin.T * (sgn * kw[swap])[:, None]).astype(np.float16)
    cosq2 = np.ascontiguousarray(np.concatenate([cosq_h, cosq_h], axis=0))
    sinq2 = np.ascontiguousarray(np.concatenate([sinq_h, sinq_h], axis=0))

    in_maps = []
    for c in range(8):
        b, g = c // 4, c % 4
        im = {
            "xT": np.ascontiguousarray(x[b].T).astype(np.float16),
            "wq": np.ascontiguousarray(Wq[:, g * 256:(g + 1) * 256]).astype(np.float16),
            "wkv": np.ascontiguousarray(
                np.concatenate([Wkv[:, g * HD:(g + 1) * HD],
                                Wkv[:, KVH * HD + g * HD: KVH * HD + (g + 1) * HD]],
                               axis=1)).astype(np.float16),
            "wo": np.ascontiguousarray(Wo[g * 256:(g + 1) * 256, :]).astype(ml_dtypes.bfloat16),
            "cosq": cosq2, "sinq": sinq2,
            "cosk": np.ascontiguousarray(cosk_h),
            "sink": np.ascontiguousarray(sink_h),
            "masks": mask_tiles.astype(ml_dtypes.bfloat16),
            "blockind": _BLOCKIND,
        }
        in_maps.append(im)

    from concourse.bass_utils import run_bass_kernel_spmd
    res = run_bass_kernel_spmd(nc, in_maps, core_ids=list(range(8)), trace=False)

    out = np.zeros((B, S, DIM), dtype=np.float32)
    for c in range(8):
        out[c // 4] += res.results[c]["outT"].T.astype(np.float32)
    return out
